# revision 1
# baseline (speedup 1.0000x reference)
"""Echo State Network Bass kernel for Trainium2 (8 NeuronCores, SPMD).

Problem: x [B=32, C=4, T=512, I=64], input_weights Wi [R=1024, C*I=256],
reservoir_weights W [R=1024, R]. Output [B, C, T, R] f32.

    u_t = flatten(x[:,:,t,:]) @ Wi.T                     (broadcast over C)
    h_{t+1} = 0.5*tanh(u_t + h_t @ W) + 0.5*h_t          (per (b, c) row)

Sharding: data-parallel over batch, 4 batches/core. Each core handles
ROWS = 4b*4c = 16 independent reservoir rows; the time recurrence is local.

Device algorithm (per core), all fp32 (CoreSim cost model: ~2.33 us/step,
1.19 ms total):
  * s_t := 2*h_t so the update is s' = tanh(u + s@(W/2)) + 0.5*s
    (W scaled by 0.5 once at load).
  * State is kept transposed (reservoir dim on partitions) as FOUR quarter
    tiles per parity, so the matmul output lands directly in the layout the
    next step consumes (no per-step transpose) and each quarter's
    add-u -> tanh -> blend chain overlaps the PE work of later quarters.
  * PE: 64 (LDWEIGHTS+MATMUL) pairs per step: stationary = W' chunk
    [128,128], moving = s.T chunk [128,16], accumulated over k into one of 4
    PSUM quarter-banks. Within each quarter the k-loop is interleaved across
    its two m-blocks with k ascending, so the dependency on the previous
    step's LAST state quarter lands ~12 matmuls into the window and the
    chain latency is hidden.
  * 0.5*s_t is computed once per step (in parallel with the matmuls): it is
    both the blend operand AND output t-1, written directly into the
    transpose staging buffer (outputs are shifted one step; slab p holds
    t = 2p-1, 2p, plus one tail slab for t = T-1).
  * U = x @ Wi.T precomputed on-device once (PE), stored in SBUF
    transposed; added per-quarter with channel-broadcast (stride-0) DVE adds
    from a per-pair staged slice.
  * Output staging is de-transposed every 2 steps by one DVE 32x32
    block-transpose over [128, 256] and written as one contiguous 128 KB
    DMA; the host undoes the coarse block permutation while unsharding.
  * The T-loop is a For_i hardware loop (unroll 2); each dynamic
    (register-offset) AP feeds exactly one instruction to stay within
    Tile's register-lowering limits. A post-pass splits multi-sync-wait
    instructions for this walrus build.
"""

import os
import sys

import numpy as np

sys.path.insert(0, "/opt/trn_rl_repo")

from contextlib import ExitStack

import concourse.bass as bass
import concourse.tile as tile
from concourse import mybir
from concourse.masks import make_identity

F32 = mybir.dt.float32
F16 = mybir.dt.float16
AF = mybir.ActivationFunctionType
ALU = mybir.AluOpType


def _patched_drain_and_barrier(self, tick_clock, wait_clock):
    # The stock kernel-tail drain carries one sync-wait per touched semaphore;
    # this walrus build caps sync waits per TPB_CTRL instruction, so chunk the
    # waits across several sequential drains on the sync engine.
    from concourse.vector_clock import ScopedClock

    nc = self.nc
    carrier = nc.sync.drain()
    wait_clock.add_sem_waits(
        carrier.ins, ScopedClock({None: tick_clock.global_clock})
    )
    si = carrier.ins.sync_info
    waits = list(si.on_wait) if si is not None else []
    if len(waits) > 1:
        carrier.ins.sync_info.on_wait = waits[:1]
        for w in waits[1:]:
            d2 = nc.sync.drain()
            d2.ins.sync_info = mybir.SyncInfo(on_wait=[w], on_update=[])
    nc.all_engine_barrier()
    popped = nc._tile_sem_poison_stack.pop()
    assert popped is self._sem_poison
    nc.clear_and_free_semaphores(list(self.sems.allocated().values()))
    nc.all_engine_barrier()


tile.TileContext._drain_and_barrier = _patched_drain_and_barrier

_MAX_SYNC_WAITS = 1


def _split_sync_waits(nc):
    """This walrus build rejects instructions carrying more than a couple of
    sync waits. Move excess waits onto same-engine NoOp carriers inserted
    immediately before the instruction (sem thresholds are absolute, so
    waiting earlier in the same engine stream is equivalent)."""
    import copy

    # template NoOp built through a scratch Bass so we never emit into nc
    scratch = bass.Bass("TRN2", target_bir_lowering=False, debug=False)
    with scratch.Block() as blk:

        @blk.sync
        def _(sync):
            sync.nop(hint="waitcarrier")

    template = None
    for bb in scratch.m.functions[0].blocks:
        for i in bb.instructions:
            if i.opcode == "NoOp":
                template = i
    assert template is not None

    n_added = 0
    for f in nc.m.functions:
        for bb in f.blocks:
            out = []
            for inst in bb.instructions:
                si = inst.sync_info
                waits = list(si.on_wait) if si is not None else []
                if len(waits) > _MAX_SYNC_WAITS:
                    extra = waits[: -_MAX_SYNC_WAITS]
                    for w in extra:
                        nop = copy.copy(template)
                        n_added += 1
                        nop.name = f"I-wsplit{n_added}"
                        nop.engine = inst.engine
                        nop.sync_info = mybir.SyncInfo(on_wait=[w], on_update=[])
                        out.append(nop)
                    inst.sync_info.on_wait = waits[-_MAX_SYNC_WAITS:]
                out.append(inst)
            if n_added:
                bb.instructions[:] = out
    return n_added

B, C, T, I, R = 32, 4, 512, 64, 1024
NCORES = 8
BL = B // NCORES          # 4 local batches per core
ROWS = BL * C             # 16 rows; row = b*C + c
KC = R // 128             # 8 contraction chunks
MC = R // 128             # 8 output blocks
CI = C * I                # 256
CIC = CI // 128           # 2 ci chunks


def _precompute_u(ctx, tc, x, wi, u_sb, tval):
    """U.T into SBUF: u_sb[p, t, m, b] = (x[b,:,t,:].flatten() @ Wi.T)[128*m+p]."""
    nc = tc.nc
    with (
        tc.tile_pool(name="pre", bufs=1) as pre,
        tc.tile_pool(name="pps", bufs=2, space="PSUM") as pps,
    ):
        ident = pre.tile([128, 128], F32, tag="ident")
        make_identity(nc, ident)

        # Wi natural [r-block, ci], then PE-transpose to WiT [ci, r]
        win = []
        for m in range(MC):
            t_ = pre.tile([128, CI], F32, tag=f"win{m}", name=f"win{m}")
            nc.sync.dma_start(out=t_, in_=wi[128 * m : 128 * (m + 1), :])
            win.append(t_)
        wiT = [pre.tile([128, R], F32, tag=f"wit{c}", name=f"wit{c}") for c in range(CIC)]
        for m in range(MC):
            for ck in range(CIC):
                pt = pps.tile([128, 128], F32, tag="pt")
                nc.tensor.transpose(pt, win[m][:, 128 * ck : 128 * (ck + 1)], ident)
                nc.scalar.copy(out=wiT[ck][:, 128 * m : 128 * (m + 1)], in_=pt)

        # xT [(c2 i) part, (t, b) free] per ci chunk
        xT = [pre.tile([128, tval, BL], F32, tag=f"xt{c}", name=f"xt{c}") for c in range(CIC)]
        for ck in range(CIC):
            for cv in range(2):
                for b in range(BL):
                    src = x[b, 2 * ck + cv, :, :].rearrange("t i -> i t")
                    nc.sync.dma_start(
                        out=xT[ck][I * cv : I * (cv + 1), :, b], in_=src
                    )

        # U.T = WiT.T @ xT, 512-wide n-blocks
        ntb = tval * BL
        nb_sz = 512
        nblocks = (ntb + nb_sz - 1) // nb_sz
        for nb in range(nblocks):
            cnt = min(nb_sz, ntb - nb * nb_sz)
            tspan = cnt // BL
            t0 = nb * nb_sz // BL
            for m in range(MC):
                psu = pps.tile([128, nb_sz], F32, tag="psu")
                for ck in range(CIC):
                    nc.tensor.matmul(
                        psu[:, :cnt],
                        wiT[ck][:, 128 * m : 128 * (m + 1)],
                        xT[ck].rearrange("p t b -> p (t b)")[
                            :, nb * nb_sz : nb * nb_sz + cnt
                        ],
                        start=(ck == 0),
                        stop=(ck == CIC - 1),
                    )
                nc.scalar.copy(
                    out=u_sb[:, t0 : t0 + tspan, m, :],
                    in_=psu[:, :cnt].rearrange("p (t b) -> p t b", b=BL),
                )


def esn_kernel(ctx, tc, x, wi, w, out, tval, dynamic, ct4=False, reps=1, npairs_override=None):
    nc = tc.nc
    consts = ctx.enter_context(tc.tile_pool(name="consts", bufs=1))

    # matmul operand dtype: fp32 (safe default) or fp16 (4x cheaper PE per
    # the cost model; ~1.7e-4 rel err from 10-bit-mantissa quantization)
    sdt = F16 if os.environ.get("ESN_DTYPE") == "fp16" else F32

    # W' = W/2, resident: 8 tiles [128, 1024]
    w_tiles = []
    for k in range(KC):
        wt = consts.tile([128, R], F32, tag=f"w{k}", name=f"w{k}")
        nc.sync.dma_start(out=wt, in_=w[128 * k : 128 * (k + 1), :])
        nc.vector.tensor_scalar_mul(wt, wt, 0.5)
        if sdt is F16:
            wt16 = consts.tile([128, R], F16, tag=f"wh{k}", name=f"wh{k}")
            nc.vector.tensor_copy(wt16, wt)
            wt = wt16
        w_tiles.append(wt)

    # U in SBUF, transposed layout
    u_sb = consts.tile([128, tval, MC, BL], F32, tag="usb")
    _precompute_u(ctx, tc, x, wi, u_sb, tval)

    ppool = ctx.enter_context(tc.tile_pool(name="ps", bufs=2, space="PSUM"))
    gpool = ctx.enter_context(tc.tile_pool(name="g", bufs=2))
    trpool = ctx.enter_context(tc.tile_pool(name="tr", bufs=2))
    spool = ctx.enter_context(tc.tile_pool(name="s", bufs=1))

    # state s = 2h as 4 quarter tiles per parity, so the next step's
    # matmuls can start on quarter 0 while later quarters' chains finish
    sQ = [
        [
            spool.tile([128, 2 * ROWS], sdt, tag=f"sQ{j}_{qd}", name=f"sQ{j}_{qd}")
            for qd in range(4)
        ]
        for j in range(2)
    ]
    for qd in range(4):
        nc.vector.memset(sQ[0][qd], 0.0)

    def step(iv, q, s_cur, s_new, trbuf, ut2):
        # t = 2*iv + q. s_cur/s_new are lists of 4 quarter tiles.
        g = gpool.tile([128, MC * ROWS], F32, tag="g")
        # h_t = 0.5*s_t is BOTH the blend operand for this step and the
        # t-1 output: write it straight into the transpose buffer (strided
        # per-quarter views) -- no separate staging copy needed.
        trv = trbuf.rearrange("p (m q2 row) -> p m q2 row", q2=2, row=ROWS)
        for qd in range(4):
            nc.vector.tensor_scalar_mul(
                trv[:, 2 * qd : 2 * (qd + 1), q, :],
                s_cur[qd].rearrange("p (m row) -> p m row", row=ROWS),
                0.5,
            )

        QM = MC // 4
        QF = QM * ROWS
        pszs = [
            ppool.tile([128, QF], F32, tag=f"psq{qd}", name=f"psq{qd}")
            for qd in range(4)
        ]
        # Phase schedule: each quarter runs k=0..5 first; its k=6,7 matmuls
        # (which need the PREVIOUS step's last s-quarter) are deferred past
        # the next quarter's k=0..5, giving every cross-step dependency
        # positive slack. Accumulation groups interleave across PSUM banks.
        phases = [
            (0, range(0, 6)), (1, range(0, 6)), (0, (6, 7)), (1, (6, 7)),
            (2, range(0, 6)), (2, (6, 7)), (3, range(0, 6)), (3, (6, 7)),
        ]
        done = [0] * 4
        for quad, ks in phases:
            psz = pszs[quad]
            for k in ks:
                for m in range(QM):
                    nc.tensor.matmul(
                        psz[:, ROWS * m : ROWS * (m + 1)],
                        w_tiles[k][
                            :,
                            128 * (quad * QM + m) : 128 * (quad * QM + m + 1),
                        ],
                        s_cur[k // 2][:, ROWS * (k % 2) : ROWS * (k % 2 + 1)],
                        start=(k == 0 and m == 0),
                        stop=(k == KC - 1 and m == QM - 1),
                    )
            done[quad] += len(tuple(ks))
            if done[quad] < KC:
                continue
            # quarter complete: z += u (channel-broadcast via stride-0 dim,
            # psz free idx = 16m + 4b + c), then tanh and the blend
            uv = ut2[:, q : q + 1, :, :]
            pzv = psz.rearrange("p (o m b c) -> p o m b c", o=1, b=BL, c=C)
            uvh = uv[:, :, quad * QM : (quad + 1) * QM, :]
            uv5 = bass.AP(uvh.tensor, uvh.offset, list(uvh.ap) + [[0, C]])
            nc.vector.tensor_add(pzv, pzv, uv5)
            sl = slice(quad * QF, (quad + 1) * QF)
            nc.scalar.activation(g[:, sl], psz, AF.Tanh)
            nc.vector.tensor_add(
                s_new[quad].rearrange("p (m row) -> p m row", row=ROWS),
                g[:, sl].rearrange("p (m row) -> p m row", row=ROWS),
                trv[:, 2 * quad : 2 * (quad + 1), q, :],
            )
    def pair_body(iv):
        trbuf = trpool.tile([128, 2 * MC * ROWS], F32, tag="trbuf")
        # one dynamic read per pair: stage u for both parities (a dynamic AP
        # may only feed ONE instruction -- Tile register-lowering limitation)
        ut2 = gpool.tile([128, 2, MC, BL], F32, tag="ut", name="ut")
        if dynamic:
            nc.vector.tensor_copy(ut2, u_sb[:, bass.ds(iv * 2, 2), :, :])
        else:
            nc.vector.tensor_copy(ut2, u_sb[:, 2 * iv : 2 * iv + 2, :, :])
        step(iv, 0, sQ[0], sQ[1], trbuf, ut2)
        step(iv, 1, sQ[1], sQ[0], trbuf, ut2)
        trT = trpool.tile([128, 2 * MC * ROWS], F32, tag="trT")
        nc.vector.transpose(trT, trbuf)
        # trT[32*pg + 16*q + row, 32*m + i] = h_{2iv+q}[row, 128*m + 32*pg + i]
        # out is [T/2, 128, 256]: one contiguous 128KB DMA; host undoes the
        # block permutation during unsharding.
        if dynamic:
            dst = out[bass.ts(iv, 1), :, :]
        else:
            dst = out[iv : iv + 1, :, :]
        nc.sync.dma_start(out=dst, in_=trT)

    npairs = tval // 2 if npairs_override is None else npairs_override
    loop_mode = os.environ.get("ESN_LOOP", "unroll")
    for _rep in range(reps):
        if dynamic and loop_mode == "stag" and npairs % 4 == 0:
            with tc.For_i(
                0,
                npairs // 4,
                1,
                staggered_reset=True,
                hint_engines=(mybir.EngineType.PE,),
            ) as li:
                for j in range(4):
                    pair_body(li * 4 + j)
        elif dynamic:
            tc.For_i_unrolled_general(
                0,
                npairs,
                1,
                lambda iv0, u: [pair_body(iv0 + j) for j in range(u)],
                max_unroll=int(os.environ.get("ESN_UNROLL", "2")),
                hint_engines=(mybir.EngineType.PE,),
            )
        else:
            for iv in range(npairs):
                pair_body(iv)
    # tail: output t = tval-1 is 0.5*s_final, staged via one extra slab
    trbuf = trpool.tile([128, 2 * MC * ROWS], F32, tag="trbuf", name="trbuf_tail")
    nc.vector.memset(trbuf, 0.0)
    trv = trbuf.rearrange("p (m q2 row) -> p m q2 row", q2=2, row=ROWS)
    for qd in range(4):
        nc.vector.tensor_scalar_mul(
            trv[:, 2 * qd : 2 * (qd + 1), 0, :],
            sQ[0][qd].rearrange("p (m row) -> p m row", row=ROWS),
            0.5,
        )
    trT = trpool.tile([128, 2 * MC * ROWS], F32, tag="trT", name="trT_tail")
    nc.vector.transpose(trT, trbuf)
    nc.sync.dma_start(out=out[npairs : npairs + 1, :, :], in_=trT)


def build_nc(tval=T, dynamic=True, ct4=False, reps=1, npairs_override=None):
    nc = bass.Bass("TRN2", target_bir_lowering=False, debug=False)
    x_t = nc.dram_tensor("x", [BL, C, tval, I], F32, kind="ExternalInput")
    wi_t = nc.dram_tensor("wi", [R, CI], F32, kind="ExternalInput")
    w_t = nc.dram_tensor("w", [R, R], F32, kind="ExternalInput")
    out_t = nc.dram_tensor("out", [tval // 2 + 1, 128, 2 * MC * ROWS], F32, kind="ExternalOutput")
    with tile.TileContext(nc) as tc, ExitStack() as ctx:
        esn_kernel(ctx, tc, x_t.ap(), wi_t.ap(), w_t.ap(), out_t.ap(), tval, dynamic, ct4=ct4, reps=reps, npairs_override=npairs_override)
    return nc


def unscramble(arr, tval):
    """[T/2, 128, 256] device layout -> [BL, C, T, R].

    arr[pair, 32*pg + 16*q + row, 32*m + i] = h[row, 2*pair + q, 128*m + 32*pg + i]
    with row = b*C + c.
    """
    a = arr.reshape(tval // 2 + 1, 4, 2, ROWS, MC, 32)
    a = a.transpose(3, 0, 2, 4, 1, 5)
    a = a.reshape(ROWS, tval + 2, R)[:, 1 : tval + 1]
    return np.ascontiguousarray(a).reshape(BL, C, tval, R)


def kernel(x, input_weights, reservoir_weights):
    x = np.ascontiguousarray(np.asarray(x, dtype=np.float32))
    wi = np.ascontiguousarray(np.asarray(input_weights, dtype=np.float32))
    w = np.ascontiguousarray(np.asarray(reservoir_weights, dtype=np.float32))

    from concourse.bass_utils import run_bass_kernel_spmd

    nc = build_nc(T, dynamic=True)
    _split_sync_waits(nc)
    in_maps = [
        {"x": x[BL * c : BL * (c + 1)], "wi": wi, "w": w} for c in range(NCORES)
    ]
    res = run_bass_kernel_spmd(nc, in_maps, core_ids=list(range(NCORES)))
    outs = [unscramble(np.asarray(m["out"]), T) for m in res.results]
    return np.concatenate(outs, axis=0)


if __name__ == "__main__":
    import jax

    with jax.default_device(jax.devices("cpu")[0]):
        import reference

        inputs = reference.setup_inputs()
        expected = np.asarray(reference.reference(**inputs))
    actual = kernel(**{k: np.asarray(v) for k, v in inputs.items()})
    err = np.abs(actual - expected).max()
    rel = err / max(1e-30, np.abs(expected).max())
    print(f"absmax err {err:.3e}  rel {rel:.3e}")

